# revision 35
# baseline (speedup 1.0000x reference)
"""Trainium2 Bass kernel for nn_GNN_37615323579234 (gnn_message_passing).

Math (reference, N=8192, D=64, 4 layers; layer-3 A@H products are dead code):
    l=0..3:  H_cl = relu(X1@w1+b1) + relu(X2@w2+b2);  H_ue = relu(Xue@w3+b3)
             X1 = A_cl@H_cl;  X2 = A_ue@H_ue;  Xue = A_ue@H_cl
    out = relu(colsum(H_cl3) @ Qw1 + Qb1) @ Qw2 + Qb2      # [1,1]

Strategy: row-shard A_cl/A_ue over 8 cores (1024 rows each).  Host pre-scales
A by 2^13 and casts to fp8 e4m3 so each core's A^T block pair is 16 MiB —
SBUF-RESIDENT, DMA'd from HBM exactly once while layer 0 computes.  H is
stored x2^6 in fp8 so the big matmuls run DoubleRow (2 k-tiles / instruction).
All scales are powers of two folded exactly into f32 weights host-side.

Latency structure (the HW findings that shaped it):
- PE HAM clock-gate: the PE runs at 1.2 GHz until ~3.4us of *continuous*
  activity, then 2.4 GHz until a ~3.4us idle gap.  Warm-up matmuls at t=0 and
  keep-warm matmuls across collective gaps keep every real matmul at 2.4 GHz.
- The A load is split by column half: pass A (output rows 0:512) consumes the
  first 8 MiB, finishes ~20us before the full load, and triggers its
  AllGather early; pass B (rows 512:1024) runs from SBUF under AllGather A.
- Inter-layer AllGathers are split in 2 halves (fp8, p-major layout, 1 KiB
  descriptors); each half's ~15us latency is hidden under the other half's
  matmul work in the next layer (alpha = gathered-first k-tiles, beta = rest).
"""

import os
import sys

for _p in ("/opt/trn_rl_repo", "/root/.axon_site/_ro/trn_rl_repo"):
    if os.path.isdir(_p) and _p not in sys.path:
        sys.path.insert(0, _p)

import numpy as np

N = 8192
D = 64
M = 8          # cores
R = N // M     # 1024 rows per core
P = 128        # partitions
KT = N // P    # 64 k-tiles
JT = R // P    # 8 row-tiles per core
KB = 4         # k-tiles per A-load DMA chunk (512 KiB per column half)
HC = 512       # column half width

SA = 2.0 ** 13  # A storage scale (entries ~ uniform[0, 1/8192])
SH = 2.0 ** 6   # H storage scale

NWARM = int(os.environ.get("KWARM", "28"))   # t=0 HAM warm-up matmuls
NKEEP = int(os.environ.get("KKEEP", "16"))   # keep-warm matmuls per gap

LAST_EXEC_NS = None
LAST_PROFILE = None

_CACHED = None  # compile once per process


def _build_module():
    import concourse.bacc as bacc
    import concourse.mybir as mybir
    from concourse import tile

    f32 = mybir.dt.float32
    bf16 = mybir.dt.bfloat16
    fp8 = mybir.dt.float8e4
    RELU = mybir.ActivationFunctionType.Relu
    ADD = mybir.AluOpType.add
    BYPASS = mybir.AluOpType.bypass
    DR = mybir.MatmulPerfMode.DoubleRow

    nc = bacc.Bacc(
        "TRN2",
        target_bir_lowering=False,
        debug=False,
        enable_asserts=False,
        num_devices=M,
    )

    # ---- I/O -------------------------------------------------------------
    # A^T blocks, fp8: [p, h, k, m, r'] = A_m[c*R + h*HC + r', k*P+p] * SA
    Aall_d = nc.dram_tensor("Aall", [P, 2, KT, 2, HC], fp8, kind="ExternalInput")
    # layer-0 fused inputs: rows 0-1 X1^T, 2-3 X2^T, 4-5 Xue^T, 6 ones
    Xcat_d = nc.dram_tensor("Xcat", [7, N], bf16, kind="ExternalInput")
    # layer-0 fused weights (block-diagonal + bias row), output scale SH
    Wcat_d = nc.dram_tensor("Wcat", [7, 3 * D], bf16, kind="ExternalInput")
    w1x_d = nc.dram_tensor("w1x", [D + 1, 3, D], bf16, kind="ExternalInput")
    w2x_d = nc.dram_tensor("w2x", [D + 1, 3, D], bf16, kind="ExternalInput")
    w3x_d = nc.dram_tensor("w3x", [D + 1, 3, D], bf16, kind="ExternalInput")
    q1x_d = nc.dram_tensor("q1x", [D + 1, D], f32, kind="ExternalInput")
    q2x_d = nc.dram_tensor("q2x", [D + 1, 1], f32, kind="ExternalInput")
    out_d = nc.dram_tensor("out", [1, 1], f32, kind="ExternalOutput")

    # internal DRAM for collectives (fp8 H half-blocks, p-major per rank)
    LgA = nc.dram_tensor("LgA", [P, JT // 2, 2 * D], fp8)
    LgB = nc.dram_tensor("LgB", [P, JT // 2, 2 * D], fp8)
    GgA = nc.dram_tensor("GgA", [M, P, JT // 2, 2 * D], fp8, addr_space="Shared")
    GgB = nc.dram_tensor("GgB", [M, P, JT // 2, 2 * D], fp8, addr_space="Shared")
    prd_l = nc.dram_tensor("prd_l", [D, 1], f32)
    Gpool = nc.dram_tensor("Gpool", [M, D, 1], f32, addr_space="Shared")
    LgD = nc.dram_tensor("LgD", [1, 4], f32)
    GgD = nc.dram_tensor("GgD", [M, 4], f32, addr_space="Shared")

    groups = [list(range(M))]
    nocc = bool(int(os.environ.get("KNOCC", "0")))  # no collectives (timing)

    # k-pair start indices by gather half: alpha = j<4 of every core block
    alpha = [c * JT + j for c in range(M) for j in (0, 2)]
    beta = [c * JT + j for c in range(M) for j in (4, 6)]

    def collective(op, alu, ins, outs, nocc_out):
        if nocc:
            nc.sync.dma_start(out=nocc_out, in_=ins)
        else:
            nc.gpsimd.collective_compute(
                op, alu, replica_groups=groups,
                ins=[ins.opt()], outs=[outs.opt()],
            )

    with tile.TileContext(nc) as tc, tc.tile_pool(name="persist", bufs=1) as pp:
        # persistent SBUF state
        Abuf = pp.tile([P, 2, KT, 2, HC], fp8, tag="Abuf")  # 128 KiB/partition
        Hbuf = pp.tile([P, KT, 2 * D], fp8, tag="Hbuf")     # [:,k,0:64]=Hue
        Hb4 = Hbuf[:].rearrange("p (c j) d -> p c j d", c=M)
        w1x = pp.tile([D + 1, 3, D], bf16, tag="w1xs")
        w2x = pp.tile([D + 1, 3, D], bf16, tag="w2xs")
        w3x = pp.tile([D + 1, 3, D], bf16, tag="w3xs")
        q1x = pp.tile([D + 1, D], f32, tag="q1xs")
        q2x = pp.tile([D + 1, 1], f32, tag="q2xs")
        ones_mv = pp.tile([P, 1], bf16, tag="ones_mv")
        ones_f = pp.tile([P, 1], f32, tag="ones_f")
        wscr = pp.tile([P, HC], bf16, tag="wscr")  # warm-up operand
        # epilogue X^T staging (ones rows written once, before the dummy
        # collective trigger occupies the gpsimd queue)
        XT1 = pp.tile([D + 1, HC], bf16, tag="xt1")
        XT2 = pp.tile([D + 1, HC], bf16, tag="xt2")
        XT3 = pp.tile([D + 1, HC], bf16, tag="xt3")

        nc.gpsimd.memset(ones_mv[:], 1.0)
        nc.gpsimd.memset(ones_f[:], 1.0)
        nc.gpsimd.memset(wscr[:], 1.0)
        nc.gpsimd.memset(XT1[D:D + 1, :], 1.0)
        nc.gpsimd.memset(XT2[D:D + 1, :], 1.0)
        nc.gpsimd.memset(XT3[D:D + 1, :], 1.0)
        # tiny throwaway AllGather: pays the ncfw barrier + first-collective
        # setup cost during phase 1 instead of at the first real gather
        if not nocc:
            nc.gpsimd.collective_compute(
                "AllGather", BYPASS, replica_groups=groups,
                ins=[LgD[:].opt()], outs=[GgD[:].opt()],
            )

        # ---- phase 0/1: warm-up + A load + layer 0 -----------------------
        with (
            tc.tile_pool(name="p0", bufs=1) as p0,
            tc.tile_pool(name="p0t", bufs=3) as p0t,
            tc.tile_pool(name="ps0", bufs=3, space="PSUM") as ps0p,
            tc.tile_pool(name="psw", bufs=1, space="PSUM") as pswp,
        ):
            Xcat = p0.tile([7, N], bf16, tag="xcat")
            Wcat = p0.tile([7, 3 * D], bf16, tag="wcat")
            nc.sync.dma_start(out=Xcat[:], in_=Xcat_d[:])
            nc.sync.dma_start(out=Wcat[:], in_=Wcat_d[:])
            nc.sync.dma_start(out=w1x[:], in_=w1x_d[:])
            nc.sync.dma_start(out=w2x[:], in_=w2x_d[:])
            nc.sync.dma_start(out=w3x[:], in_=w3x_d[:])
            nc.sync.dma_start(out=q1x[:], in_=q1x_d[:])
            nc.sync.dma_start(out=q2x[:], in_=q2x_d[:])

            # stream the A block: column half 0 first (feeds pass A)
            for h in range(2):
                for kb in range(KT // KB):
                    ksl = slice(kb * KB, (kb + 1) * KB)
                    nc.sync.dma_start(out=Abuf[:, h, ksl, :, :],
                                      in_=Aall_d[:, h, ksl, :, :])

            # HAM warm-up: ~3.4us of continuous PE work unthrottles the clock
            wps = pswp.tile([P, 3 * D], f32, tag="wps")
            nc.scalar.activation(wscr[:, 0:1], ones_mv[:], RELU)  # ACT table
            for _ in range(NWARM):
                nc.tensor.matmul(wps[:], wscr[:, 0:P], wscr[:, 0:3 * D],
                                 start=True, stop=True)

            # layer 0: H0 for all N rows (replicated on every core), fp8
            for b in range(KT // 2):
                ps0 = ps0p.tile([P, 2, 3 * D], f32, tag="ps0")
                for i in range(2):
                    k = 2 * b + i
                    sl = slice(k * P, (k + 1) * P)
                    nc.tensor.matmul(ps0[:, i, :], Xcat[:, sl], Wcat[:],
                                     start=True, stop=True)
                ksl = slice(2 * b, 2 * b + 2)
                t12 = p0t.tile([P, 2, 2 * D], bf16, tag="t12")
                nc.scalar.activation(t12[:], ps0[:, :, 0:2 * D], RELU)
                nc.vector.tensor_scalar_max(Hbuf[:, ksl, 0:D],
                                            ps0[:, :, 2 * D:3 * D], 0.0)
                nc.vector.tensor_tensor(Hbuf[:, ksl, D:2 * D],
                                        t12[:, :, 0:D], t12[:, :, D:2 * D], ADD)

        # ---- main layers -------------------------------------------------
        with (
            tc.tile_pool(name="sbE", bufs=1) as sbE,
            tc.tile_pool(name="psA", bufs=1, space="PSUM") as psA,
            tc.tile_pool(name="psE", bufs=1, space="PSUM") as psE,
        ):
            def keep_warm(n):
                # slow f32 matmuls into a dead PSUM bank bridge idle gaps so
                # HAM doesn't re-throttle; next real use starts start=True.
                pnw = psE.tile([P, JT // 2, D], f32, tag="pn1")
                for _ in range(n):
                    nc.tensor.matmul(pnw[:], wscr[:, 0:P],
                                     wscr[:, 0:2 * P], start=True, stop=True)

            def acc_mms(l, pairs, h, Pcl, Pue, s_pairs, e_pairs):
                last = l == 2
                wue = 2 * D if not last else D
                for k0 in pairs:
                    ksl = slice(k0, k0 + 2)
                    s = k0 == s_pairs
                    e = k0 == e_pairs
                    nc.tensor.matmul(Pcl[:], Hbuf[:, ksl, D:2 * D],
                                     Abuf[:, h, ksl, 0, :],
                                     start=s, stop=e, perf_mode=DR)
                    nc.tensor.matmul(Pue[:], Hbuf[:, ksl, 0:wue],
                                     Abuf[:, h, ksl, 1, :],
                                     start=s, stop=e, perf_mode=DR)

            def epilogue_half(l, hf, Pcl, Pue, Ppool=None):
                # hf: 0 = output rows 0:512 (jj 0-3), 1 = rows 512:1024
                last = l == 2
                nc.vector.tensor_copy(XT1[0:D, :], Pcl[:])
                nc.vector.tensor_copy(XT2[0:D, :], Pue[0:D, :])
                if not last:
                    nc.vector.tensor_copy(XT3[0:D, :], Pue[D:2 * D, :])
                Pn1 = psE.tile([P, JT // 2, D], f32, tag="pn1")
                Pn2 = psE.tile([P, JT // 2, D], f32, tag="pn2")
                if not last:
                    Pnue = psE.tile([P, JT // 2, D], f32, tag="pnue")
                for jj in range(JT // 2):
                    sl = slice(jj * P, (jj + 1) * P)
                    nc.tensor.matmul(Pn1[:, jj, :], XT1[:, sl], w1x[:, l, :],
                                     start=True, stop=True)
                    nc.tensor.matmul(Pn2[:, jj, :], XT2[:, sl], w2x[:, l, :],
                                     start=True, stop=True)
                    if not last:
                        nc.tensor.matmul(Pnue[:, jj, :], XT3[:, sl],
                                         w3x[:, l, :], start=True, stop=True)
                t1 = sbE.tile([P, JT // 2, D], f32, tag="t1")
                t2 = sbE.tile([P, JT // 2, D], f32, tag="t2")
                nc.scalar.activation(t1[:], Pn1[:], RELU)
                nc.scalar.activation(t2[:], Pn2[:], RELU)
                if not last:
                    Epad = sbE.tile([P, JT // 2, 2 * D], fp8,
                                    tag=f"epad{hf}")
                    nc.scalar.activation(Epad[:, :, 0:D], Pnue[:], RELU)
                    nc.vector.tensor_tensor(Epad[:, :, D:2 * D],
                                            t1[:], t2[:], ADD)
                    Lg = LgA if hf == 0 else LgB
                    Gg = GgA if hf == 0 else GgB
                    nc.sync.dma_start(out=Lg[:], in_=Epad[:])
                    collective("AllGather", BYPASS, Lg[:], Gg[:], Gg[0])
                else:
                    hs = sbE.tile([P, JT // 2, D], bf16, tag="hs")
                    nc.vector.tensor_tensor(hs[:], t1[:], t2[:], ADD)
                    for jj in range(JT // 2):
                        nc.tensor.matmul(
                            Ppool[:], hs[:, jj, :], ones_mv[:],
                            start=(hf == 0 and jj == 0),
                            stop=(hf == 1 and jj == JT // 2 - 1),
                        )

            for l in range(3):
                last = l == 2
                wue = 2 * D if not last else D
                Pcl0 = psA.tile([D, HC], f32, tag="acc_cl0")
                Pcl1 = psA.tile([D, HC], f32, tag="acc_cl1")
                Pue0 = psA.tile([wue, HC], f32, tag="acc_ue0")
                Pue1 = psA.tile([wue, HC], f32, tag="acc_ue1")
                if last:
                    Ppool = psE.tile([D, 1], f32, tag="pooled")
                else:
                    Ppool = None

                if l == 0:
                    # H0 is local; pass A is paced by the arriving A chunks
                    allp = [2 * kp for kp in range(KT // 2)]
                    acc_mms(0, allp, 0, Pcl0, Pue0, 0, KT - 2)
                    epilogue_half(0, 0, Pcl0, Pue0)
                    acc_mms(0, allp, 1, Pcl1, Pue1, 0, KT - 2)
                    epilogue_half(0, 1, Pcl1, Pue1)
                    keep_warm(NKEEP)
                else:
                    # gathered halves arrive as alpha (coll A), beta (coll B)
                    nc.sync.dma_start(
                        out=Hb4[:, :, 0:JT // 2, :],
                        in_=GgA[:].rearrange("c p j d -> p c j d"))
                    nc.sync.dma_start(
                        out=Hb4[:, :, JT // 2:JT, :],
                        in_=GgB[:].rearrange("c p j d -> p c j d"))
                    acc_mms(l, alpha, 0, Pcl0, Pue0, alpha[0], beta[-1])
                    acc_mms(l, beta, 0, Pcl0, Pue0, alpha[0], beta[-1])
                    epilogue_half(l, 0, Pcl0, Pue0, Ppool)
                    acc_mms(l, alpha, 1, Pcl1, Pue1, alpha[0], beta[-1])
                    acc_mms(l, beta, 1, Pcl1, Pue1, alpha[0], beta[-1])
                    epilogue_half(l, 1, Pcl1, Pue1, Ppool)
                    if not last:
                        keep_warm(NKEEP)

            # ---- pooled vector -> AllReduce -> head MLP ------------------
            pl_s = sbE.tile([D, 1], f32, tag="pl")
            nc.vector.tensor_copy(pl_s[:], Ppool[:])
            nc.sync.dma_start(out=prd_l[:], in_=pl_s[:])
            # AllGather of per-core pooled partials (cheaper than AllReduce),
            # then sum across cores with a tiny matmul
            collective("AllGather", BYPASS, prd_l[:], Gpool[:], Gpool[0])
            pvec = sbE.tile([D + 1, 1], f32, tag="pvec")
            zt = sbE.tile([D + 1, 1], f32, tag="zt")
            nc.gpsimd.memset(pvec[D:D + 1, :], 1.0)
            nc.gpsimd.memset(zt[D:D + 1, :], 1.0)
            pvec8 = sbE.tile([M, D], f32, tag="pvec8")
            nc.sync.dma_start(out=pvec8[:],
                              in_=Gpool[:].rearrange("c d x -> c (d x)"))
            Pp2 = psE.tile([D, 1], f32, tag="pooled")
            nc.tensor.matmul(Pp2[:], pvec8[:], ones_f[0:M, :],
                             start=True, stop=True)
            nc.vector.tensor_copy(pvec[0:D, :], Pp2[:])
            Pz = psE.tile([D, 1], f32, tag="pooled")
            nc.tensor.matmul(Pz[:], q1x[:], pvec[:], start=True, stop=True)
            nc.scalar.activation(zt[0:D, :], Pz[:], RELU)
            Po = psE.tile([1, 1], f32, tag="pooled")
            nc.tensor.matmul(Po[:], q2x[:], zt[:], start=True, stop=True)
            o_s = sbE.tile([1, 1], f32, tag="os")
            nc.vector.tensor_copy(o_s[:], Po[:])
            nc.sync.dma_start(out=out_d[:], in_=o_s[:])

    nc.compile()
    return nc


def _get_module():
    global _CACHED
    if _CACHED is None:
        _CACHED = _build_module()
    return _CACHED


def prep_in_maps(inputs):
    import ml_dtypes

    f = np.float32
    f8 = ml_dtypes.float8_e4m3
    bf = ml_dtypes.bfloat16
    A_cl = np.asarray(inputs["A_cl"], f)
    A_ue = np.asarray(inputs["A_ue"], f)
    ones_row = np.ones((1, N), f)

    Xcat = np.ascontiguousarray(np.vstack([
        np.asarray(inputs["X_cl_1"], f).T,
        np.asarray(inputs["X_cl_2"], f).T,
        np.asarray(inputs["X_ue"], f).T,
        ones_row,
    ]).astype(bf))

    # layer-0 fused block-diagonal weights, output scale SH
    Wcat = np.zeros((7, 3 * D), f)
    Wcat[0:2, 0:D] = np.asarray(inputs["W1_w0"], f) * SH
    Wcat[2:4, D:2 * D] = np.asarray(inputs["W2_w0"], f) * SH
    Wcat[4:6, 2 * D:3 * D] = np.asarray(inputs["W3_w0"], f) * SH
    Wcat[6, 0:D] = np.asarray(inputs["W1_b0"], f) * SH
    Wcat[6, D:2 * D] = np.asarray(inputs["W2_b0"], f) * SH
    Wcat[6, 2 * D:3 * D] = np.asarray(inputs["W3_b0"], f) * SH

    def wx(w, b):
        # [3, D, D] + [3, D] -> [D+1, 3, D]; input X^T carries scale SA*SH,
        # layers 1-2 re-emit H*SH, layer 3 emits unscaled H.
        w = np.asarray(w, f)
        b = np.asarray(b, f)
        cols = []
        for i in range(3):
            w_scale = (1.0 / SA) if i < 2 else (1.0 / (SA * SH))
            b_scale = SH if i < 2 else 1.0
            cols.append(np.vstack([w[i] * w_scale, b[i][None, :] * b_scale]))
        return np.ascontiguousarray(np.stack(cols, axis=1))

    common = {
        "Xcat": Xcat,
        "Wcat": np.ascontiguousarray(Wcat.astype(bf)),
        "w1x": wx(inputs["W1_w"], inputs["W1_b"]).astype(bf),
        "w2x": wx(inputs["W2_w"], inputs["W2_b"]).astype(bf),
        "w3x": wx(inputs["W3_w"], inputs["W3_b"]).astype(bf),
        "q1x": np.ascontiguousarray(
            np.vstack([np.asarray(inputs["Q_w1"], f),
                       np.asarray(inputs["Q_b1"], f)[None, :]])
        ),
        "q2x": np.ascontiguousarray(
            np.vstack([np.asarray(inputs["Q_w2"], f),
                       np.asarray(inputs["Q_b2"], f)[None, :]])
        ),
    }

    # A blocks: [p, h, k, m, r'] = A_m[c*R + h*HC + r', k*P + p] * SA, fp8
    Acl8 = (A_cl * SA).astype(f8)
    Aue8 = (A_ue * SA).astype(f8)

    in_maps = []
    for c in range(M):
        rs = slice(c * R, (c + 1) * R)
        # [R, N] -> [h, r', k, p] -> [p, h, k, r']
        acl = Acl8[rs, :].reshape(2, HC, KT, P).transpose(3, 0, 2, 1)
        aue = Aue8[rs, :].reshape(2, HC, KT, P).transpose(3, 0, 2, 1)
        m = dict(common)
        m["Aall"] = np.ascontiguousarray(
            np.stack([acl, aue], axis=3))  # [P, 2, KT, 2, HC]
        in_maps.append(m)
    return in_maps


def kernel(**inputs):
    global LAST_EXEC_NS, LAST_PROFILE
    nc = _get_module()
    from concourse.bass_utils import run_bass_kernel_spmd

    in_maps = prep_in_maps(inputs)
    res = run_bass_kernel_spmd(nc, in_maps, core_ids=list(range(M)), trace=False)
    LAST_EXEC_NS = res.exec_time_ns
    LAST_PROFILE = res.profile_json
    return np.asarray(res.results[0]["out"], np.float32)


# revision 38
# speedup vs baseline: 1.0125x; 1.0125x over previous
"""Trainium2 Bass kernel for nn_GNN_37615323579234 (gnn_message_passing).

Math (reference, N=8192, D=64, 4 layers; layer-3 A@H products are dead code):
    l=0..3:  H_cl = relu(X1@w1+b1) + relu(X2@w2+b2);  H_ue = relu(Xue@w3+b3)
             X1 = A_cl@H_cl;  X2 = A_ue@H_ue;  Xue = A_ue@H_cl
    out = relu(colsum(H_cl3) @ Qw1 + Qb1) @ Qw2 + Qb2      # [1,1]

Strategy: row-shard A_cl/A_ue over 8 cores (1024 rows each).  Host pre-scales
A by 2^13 and casts to fp8 e4m3 so each core's A^T block pair is 16 MiB —
SBUF-RESIDENT, DMA'd from HBM exactly once while layer 0 computes.  H is
stored x2^6 in fp8 so the big matmuls run DoubleRow (2 k-tiles / instruction).
All scales are powers of two folded exactly into f32 weights host-side.

Latency structure (the HW findings that shaped it):
- PE HAM clock-gate: the PE runs at 1.2 GHz until ~3.4us of *continuous*
  activity, then 2.4 GHz until a ~3.4us idle gap.  Warm-up matmuls at t=0 and
  keep-warm matmuls across collective gaps keep every real matmul at 2.4 GHz.
- The A load is split by column half: pass A (output rows 0:512) consumes the
  first 8 MiB, finishes ~20us before the full load, and triggers its
  AllGather early; pass B (rows 512:1024) runs from SBUF under AllGather A.
- Inter-layer AllGathers are split in 2 halves (fp8, p-major layout, 1 KiB
  descriptors); each half's ~15us latency is hidden under the other half's
  matmul work in the next layer (alpha = gathered-first k-tiles, beta = rest).
"""

import os
import sys

for _p in ("/opt/trn_rl_repo", "/root/.axon_site/_ro/trn_rl_repo"):
    if os.path.isdir(_p) and _p not in sys.path:
        sys.path.insert(0, _p)

import numpy as np

N = 8192
D = 64
M = 8          # cores
R = N // M     # 1024 rows per core
P = 128        # partitions
KT = N // P    # 64 k-tiles
JT = R // P    # 8 row-tiles per core
KB = 4         # k-tiles per A-load DMA chunk (512 KiB per column half)
HC = 512       # column half width

SA = 2.0 ** 13  # A storage scale (entries ~ uniform[0, 1/8192])
SH = 2.0 ** 6   # H storage scale

NWARM = int(os.environ.get("KWARM", "28"))   # t=0 HAM warm-up matmuls
NKEEP = int(os.environ.get("KKEEP", "16"))   # keep-warm matmuls per gap

LAST_EXEC_NS = None
LAST_PROFILE = None

_CACHED = None  # compile once per process


def _build_module():
    import concourse.bacc as bacc
    import concourse.mybir as mybir
    from concourse import tile

    f32 = mybir.dt.float32
    bf16 = mybir.dt.bfloat16
    fp8 = mybir.dt.float8e4
    RELU = mybir.ActivationFunctionType.Relu
    ADD = mybir.AluOpType.add
    BYPASS = mybir.AluOpType.bypass
    DR = mybir.MatmulPerfMode.DoubleRow

    nc = bacc.Bacc(
        "TRN2",
        target_bir_lowering=False,
        debug=False,
        enable_asserts=False,
        num_devices=M,
    )

    # ---- I/O -------------------------------------------------------------
    # A^T blocks, fp8: [p, h, k, m, r'] = A_m[c*R + h*HC + r', k*P+p] * SA
    Aall_d = nc.dram_tensor("Aall", [P, 2, KT, 2, HC], fp8, kind="ExternalInput")
    # layer-0 fused inputs: rows 0-1 X1^T, 2-3 X2^T, 4-5 Xue^T, 6 ones
    Xcat_d = nc.dram_tensor("Xcat", [7, N], bf16, kind="ExternalInput")
    # layer-0 fused weights (block-diagonal + bias row), output scale SH
    Wcat_d = nc.dram_tensor("Wcat", [7, 3 * D], bf16, kind="ExternalInput")
    w1x_d = nc.dram_tensor("w1x", [D + 1, 3, D], bf16, kind="ExternalInput")
    w2x_d = nc.dram_tensor("w2x", [D + 1, 3, D], bf16, kind="ExternalInput")
    w3x_d = nc.dram_tensor("w3x", [D + 1, 3, D], bf16, kind="ExternalInput")
    q1x_d = nc.dram_tensor("q1x", [D + 1, D], f32, kind="ExternalInput")
    q2x_d = nc.dram_tensor("q2x", [D + 1, 1], f32, kind="ExternalInput")
    out_d = nc.dram_tensor("out", [1, 1], f32, kind="ExternalOutput")

    # internal DRAM for collectives (fp8 H half-blocks, p-major per rank)
    LgA = nc.dram_tensor("LgA", [P, JT // 2, 2 * D], fp8)
    LgB = nc.dram_tensor("LgB", [P, JT // 2, 2 * D], fp8)
    LgF = nc.dram_tensor("LgF", [P, JT, 2 * D], fp8)
    GgF = nc.dram_tensor("GgF", [M, P, JT, 2 * D], fp8, addr_space="Shared")
    GgA = nc.dram_tensor("GgA", [M, P, JT // 2, 2 * D], fp8, addr_space="Shared")
    GgB = nc.dram_tensor("GgB", [M, P, JT // 2, 2 * D], fp8, addr_space="Shared")
    prd_l = nc.dram_tensor("prd_l", [D, 1], f32)
    Gpool = nc.dram_tensor("Gpool", [M, D, 1], f32, addr_space="Shared")
    LgD = nc.dram_tensor("LgD", [1, 4], f32)
    GgD = nc.dram_tensor("GgD", [M, 4], f32, addr_space="Shared")

    groups = [list(range(M))]
    nocc = bool(int(os.environ.get("KNOCC", "0")))  # no collectives (timing)

    # k-pair start indices by gather half: alpha = j<4 of every core block
    alpha = [c * JT + j for c in range(M) for j in (0, 2)]
    beta = [c * JT + j for c in range(M) for j in (4, 6)]

    def collective(op, alu, ins, outs, nocc_out):
        if nocc:
            nc.sync.dma_start(out=nocc_out, in_=ins)
        else:
            nc.gpsimd.collective_compute(
                op, alu, replica_groups=groups,
                ins=[ins.opt()], outs=[outs.opt()],
            )

    with tile.TileContext(nc) as tc, tc.tile_pool(name="persist", bufs=1) as pp:
        # persistent SBUF state
        Abuf = pp.tile([P, 2, KT, 2, HC], fp8, tag="Abuf")  # 128 KiB/partition
        Hbuf = pp.tile([P, KT, 2 * D], fp8, tag="Hbuf")     # [:,k,0:64]=Hue
        Hb4 = Hbuf[:].rearrange("p (c j) d -> p c j d", c=M)
        w1x = pp.tile([D + 1, 3, D], bf16, tag="w1xs")
        w2x = pp.tile([D + 1, 3, D], bf16, tag="w2xs")
        w3x = pp.tile([D + 1, 3, D], bf16, tag="w3xs")
        q1x = pp.tile([D + 1, D], f32, tag="q1xs")
        q2x = pp.tile([D + 1, 1], f32, tag="q2xs")
        ones_mv = pp.tile([P, 1], bf16, tag="ones_mv")
        ones_f = pp.tile([P, 1], f32, tag="ones_f")
        wscr = pp.tile([P, HC], bf16, tag="wscr")  # warm-up operand
        # epilogue X^T staging (ones rows written once, before the dummy
        # collective trigger occupies the gpsimd queue)
        XT1 = pp.tile([D + 1, HC], bf16, tag="xt1")
        XT2 = pp.tile([D + 1, HC], bf16, tag="xt2")
        XT3 = pp.tile([D + 1, HC], bf16, tag="xt3")

        nc.gpsimd.memset(ones_mv[:], 1.0)
        nc.gpsimd.memset(ones_f[:], 1.0)
        nc.gpsimd.memset(wscr[:], 1.0)
        nc.gpsimd.memset(XT1[D:D + 1, :], 1.0)
        nc.gpsimd.memset(XT2[D:D + 1, :], 1.0)
        nc.gpsimd.memset(XT3[D:D + 1, :], 1.0)
        # tiny throwaway AllGather: pays the ncfw barrier + first-collective
        # setup cost during phase 1 instead of at the first real gather
        if not nocc:
            nc.gpsimd.collective_compute(
                "AllGather", BYPASS, replica_groups=groups,
                ins=[LgD[:].opt()], outs=[GgD[:].opt()],
            )

        # ---- phase 0/1: warm-up + A load + layer 0 -----------------------
        with (
            tc.tile_pool(name="p0", bufs=1) as p0,
            tc.tile_pool(name="p0t", bufs=3) as p0t,
            tc.tile_pool(name="ps0", bufs=3, space="PSUM") as ps0p,
            tc.tile_pool(name="psw", bufs=1, space="PSUM") as pswp,
        ):
            Xcat = p0.tile([7, N], bf16, tag="xcat")
            Wcat = p0.tile([7, 3 * D], bf16, tag="wcat")
            nc.sync.dma_start(out=Xcat[:], in_=Xcat_d[:])
            nc.sync.dma_start(out=Wcat[:], in_=Wcat_d[:])
            nc.sync.dma_start(out=w1x[:], in_=w1x_d[:])
            nc.sync.dma_start(out=w2x[:], in_=w2x_d[:])
            nc.sync.dma_start(out=w3x[:], in_=w3x_d[:])
            nc.sync.dma_start(out=q1x[:], in_=q1x_d[:])
            nc.sync.dma_start(out=q2x[:], in_=q2x_d[:])

            # stream the A block: column half 0 first (feeds pass A)
            for h in range(2):
                for kb in range(KT // KB):
                    ksl = slice(kb * KB, (kb + 1) * KB)
                    nc.sync.dma_start(out=Abuf[:, h, ksl, :, :],
                                      in_=Aall_d[:, h, ksl, :, :])

            # HAM warm-up: ~3.4us of continuous PE work unthrottles the clock
            wps = pswp.tile([P, 3 * D], f32, tag="wps")
            nc.scalar.activation(wscr[:, 0:1], ones_mv[:], RELU)  # ACT table
            for _ in range(NWARM):
                nc.tensor.matmul(wps[:], wscr[:, 0:P], wscr[:, 0:3 * D],
                                 start=True, stop=True)

            # layer 0: H0 for all N rows (replicated on every core), fp8
            for b in range(KT // 2):
                ps0 = ps0p.tile([P, 2, 3 * D], f32, tag="ps0")
                for i in range(2):
                    k = 2 * b + i
                    sl = slice(k * P, (k + 1) * P)
                    nc.tensor.matmul(ps0[:, i, :], Xcat[:, sl], Wcat[:],
                                     start=True, stop=True)
                ksl = slice(2 * b, 2 * b + 2)
                t12 = p0t.tile([P, 2, 2 * D], bf16, tag="t12")
                nc.scalar.activation(t12[:], ps0[:, :, 0:2 * D], RELU)
                nc.vector.tensor_scalar_max(Hbuf[:, ksl, 0:D],
                                            ps0[:, :, 2 * D:3 * D], 0.0)
                nc.vector.tensor_tensor(Hbuf[:, ksl, D:2 * D],
                                        t12[:, :, 0:D], t12[:, :, D:2 * D], ADD)

        # ---- main layers -------------------------------------------------
        with (
            tc.tile_pool(name="sbE", bufs=1) as sbE,
            tc.tile_pool(name="psA", bufs=1, space="PSUM") as psA,
            tc.tile_pool(name="psE", bufs=1, space="PSUM") as psE,
        ):
            def keep_warm(n):
                # slow f32 matmuls into a dead PSUM bank bridge idle gaps so
                # HAM doesn't re-throttle; next real use starts start=True.
                pnw = psE.tile([P, JT // 2, D], f32, tag="pn1")
                for _ in range(n):
                    nc.tensor.matmul(pnw[:], wscr[:, 0:P],
                                     wscr[:, 0:2 * P], start=True, stop=True)

            def acc_mms(l, pairs, h, Pcl, Pue, s_pairs, e_pairs):
                last = l == 2
                wue = 2 * D if not last else D
                for k0 in pairs:
                    ksl = slice(k0, k0 + 2)
                    s = k0 == s_pairs
                    e = k0 == e_pairs
                    nc.tensor.matmul(Pcl[:], Hbuf[:, ksl, D:2 * D],
                                     Abuf[:, h, ksl, 0, :],
                                     start=s, stop=e, perf_mode=DR)
                    nc.tensor.matmul(Pue[:], Hbuf[:, ksl, 0:wue],
                                     Abuf[:, h, ksl, 1, :],
                                     start=s, stop=e, perf_mode=DR)

            def epilogue_half(l, hf, Pcl, Pue, Ppool=None):
                # hf: 0 = output rows 0:512 (jj 0-3), 1 = rows 512:1024
                last = l == 2
                nc.vector.tensor_copy(XT1[0:D, :], Pcl[:])
                nc.vector.tensor_copy(XT2[0:D, :], Pue[0:D, :])
                if not last:
                    nc.vector.tensor_copy(XT3[0:D, :], Pue[D:2 * D, :])
                Pn1 = psE.tile([P, JT // 2, D], f32, tag="pn1")
                Pn2 = psE.tile([P, JT // 2, D], f32, tag="pn2")
                if not last:
                    Pnue = psE.tile([P, JT // 2, D], f32, tag="pnue")
                for jj in range(JT // 2):
                    sl = slice(jj * P, (jj + 1) * P)
                    nc.tensor.matmul(Pn1[:, jj, :], XT1[:, sl], w1x[:, l, :],
                                     start=True, stop=True)
                    nc.tensor.matmul(Pn2[:, jj, :], XT2[:, sl], w2x[:, l, :],
                                     start=True, stop=True)
                    if not last:
                        nc.tensor.matmul(Pnue[:, jj, :], XT3[:, sl],
                                         w3x[:, l, :], start=True, stop=True)
                t1 = sbE.tile([P, JT // 2, D], f32, tag="t1")
                t2 = sbE.tile([P, JT // 2, D], f32, tag="t2")
                nc.scalar.activation(t1[:], Pn1[:], RELU)
                nc.scalar.activation(t2[:], Pn2[:], RELU)
                if not last:
                    Epad = sbE.tile([P, JT // 2, 2 * D], fp8,
                                    tag=f"epad{hf}")
                    nc.scalar.activation(Epad[:, :, 0:D], Pnue[:], RELU)
                    nc.vector.tensor_tensor(Epad[:, :, D:2 * D],
                                            t1[:], t2[:], ADD)
                    if l == 0:
                        # L1 boundary: no compute left to overlap (ncfw
                        # barrier gates it anyway) -> one full gather beats
                        # two serial half gathers
                        jsl = slice(hf * (JT // 2), (hf + 1) * (JT // 2))
                        nc.sync.dma_start(out=LgF[:, jsl, :], in_=Epad[:])
                        if hf == 1:
                            collective("AllGather", BYPASS, LgF[:], GgF[:],
                                       GgF[0])
                    else:
                        Lg = LgA if hf == 0 else LgB
                        Gg = GgA if hf == 0 else GgB
                        nc.sync.dma_start(out=Lg[:], in_=Epad[:])
                        collective("AllGather", BYPASS, Lg[:], Gg[:], Gg[0])
                else:
                    hs = sbE.tile([P, JT // 2, D], bf16, tag="hs")
                    nc.vector.tensor_tensor(hs[:], t1[:], t2[:], ADD)
                    for jj in range(JT // 2):
                        nc.tensor.matmul(
                            Ppool[:], hs[:, jj, :], ones_mv[:],
                            start=(hf == 0 and jj == 0),
                            stop=(hf == 1 and jj == JT // 2 - 1),
                        )

            for l in range(3):
                last = l == 2
                wue = 2 * D if not last else D
                Pcl0 = psA.tile([D, HC], f32, tag="acc_cl0")
                Pcl1 = psA.tile([D, HC], f32, tag="acc_cl1")
                Pue0 = psA.tile([wue, HC], f32, tag="acc_ue0")
                Pue1 = psA.tile([wue, HC], f32, tag="acc_ue1")
                if last:
                    Ppool = psE.tile([D, 1], f32, tag="pooled")
                else:
                    Ppool = None

                if l == 0:
                    # H0 is local; pass A is paced by the arriving A chunks
                    allp = [2 * kp for kp in range(KT // 2)]
                    acc_mms(0, allp, 0, Pcl0, Pue0, 0, KT - 2)
                    epilogue_half(0, 0, Pcl0, Pue0)
                    acc_mms(0, allp, 1, Pcl1, Pue1, 0, KT - 2)
                    epilogue_half(0, 1, Pcl1, Pue1)
                    keep_warm(NKEEP)
                else:
                    # gathered halves arrive as alpha (coll A), beta (coll B)
                    if l == 1:
                        nc.sync.dma_start(
                            out=Hb4[:, :, 0:JT, :],
                            in_=GgF[:].rearrange("c p j d -> p c j d"))
                    else:
                        nc.sync.dma_start(
                            out=Hb4[:, :, 0:JT // 2, :],
                            in_=GgA[:].rearrange("c p j d -> p c j d"))
                        nc.sync.dma_start(
                            out=Hb4[:, :, JT // 2:JT, :],
                            in_=GgB[:].rearrange("c p j d -> p c j d"))
                    acc_mms(l, alpha, 0, Pcl0, Pue0, alpha[0], beta[-1])
                    acc_mms(l, beta, 0, Pcl0, Pue0, alpha[0], beta[-1])
                    epilogue_half(l, 0, Pcl0, Pue0, Ppool)
                    acc_mms(l, alpha, 1, Pcl1, Pue1, alpha[0], beta[-1])
                    acc_mms(l, beta, 1, Pcl1, Pue1, alpha[0], beta[-1])
                    epilogue_half(l, 1, Pcl1, Pue1, Ppool)
                    if not last:
                        keep_warm(NKEEP)

            # ---- pooled vector -> AllReduce -> head MLP ------------------
            pl_s = sbE.tile([D, 1], f32, tag="pl")
            nc.vector.tensor_copy(pl_s[:], Ppool[:])
            nc.sync.dma_start(out=prd_l[:], in_=pl_s[:])
            # AllGather of per-core pooled partials (cheaper than AllReduce),
            # then sum across cores with a tiny matmul
            collective("AllGather", BYPASS, prd_l[:], Gpool[:], Gpool[0])
            pvec = sbE.tile([D + 1, 1], f32, tag="pvec")
            zt = sbE.tile([D + 1, 1], f32, tag="zt")
            nc.gpsimd.memset(pvec[D:D + 1, :], 1.0)
            nc.gpsimd.memset(zt[D:D + 1, :], 1.0)
            pvec8 = sbE.tile([M, D], f32, tag="pvec8")
            nc.sync.dma_start(out=pvec8[:],
                              in_=Gpool[:].rearrange("c d x -> c (d x)"))
            Pp2 = psE.tile([D, 1], f32, tag="pooled")
            nc.tensor.matmul(Pp2[:], pvec8[:], ones_f[0:M, :],
                             start=True, stop=True)
            nc.vector.tensor_copy(pvec[0:D, :], Pp2[:])
            Pz = psE.tile([D, 1], f32, tag="pooled")
            nc.tensor.matmul(Pz[:], q1x[:], pvec[:], start=True, stop=True)
            nc.scalar.activation(zt[0:D, :], Pz[:], RELU)
            Po = psE.tile([1, 1], f32, tag="pooled")
            nc.tensor.matmul(Po[:], q2x[:], zt[:], start=True, stop=True)
            o_s = sbE.tile([1, 1], f32, tag="os")
            nc.vector.tensor_copy(o_s[:], Po[:])
            nc.sync.dma_start(out=out_d[:], in_=o_s[:])

    nc.compile()
    return nc


def _get_module():
    global _CACHED
    if _CACHED is None:
        _CACHED = _build_module()
    return _CACHED


def prep_in_maps(inputs):
    import ml_dtypes

    f = np.float32
    f8 = ml_dtypes.float8_e4m3
    bf = ml_dtypes.bfloat16
    A_cl = np.asarray(inputs["A_cl"], f)
    A_ue = np.asarray(inputs["A_ue"], f)
    ones_row = np.ones((1, N), f)

    Xcat = np.ascontiguousarray(np.vstack([
        np.asarray(inputs["X_cl_1"], f).T,
        np.asarray(inputs["X_cl_2"], f).T,
        np.asarray(inputs["X_ue"], f).T,
        ones_row,
    ]).astype(bf))

    # layer-0 fused block-diagonal weights, output scale SH
    Wcat = np.zeros((7, 3 * D), f)
    Wcat[0:2, 0:D] = np.asarray(inputs["W1_w0"], f) * SH
    Wcat[2:4, D:2 * D] = np.asarray(inputs["W2_w0"], f) * SH
    Wcat[4:6, 2 * D:3 * D] = np.asarray(inputs["W3_w0"], f) * SH
    Wcat[6, 0:D] = np.asarray(inputs["W1_b0"], f) * SH
    Wcat[6, D:2 * D] = np.asarray(inputs["W2_b0"], f) * SH
    Wcat[6, 2 * D:3 * D] = np.asarray(inputs["W3_b0"], f) * SH

    def wx(w, b):
        # [3, D, D] + [3, D] -> [D+1, 3, D]; input X^T carries scale SA*SH,
        # layers 1-2 re-emit H*SH, layer 3 emits unscaled H.
        w = np.asarray(w, f)
        b = np.asarray(b, f)
        cols = []
        for i in range(3):
            w_scale = (1.0 / SA) if i < 2 else (1.0 / (SA * SH))
            b_scale = SH if i < 2 else 1.0
            cols.append(np.vstack([w[i] * w_scale, b[i][None, :] * b_scale]))
        return np.ascontiguousarray(np.stack(cols, axis=1))

    common = {
        "Xcat": Xcat,
        "Wcat": np.ascontiguousarray(Wcat.astype(bf)),
        "w1x": wx(inputs["W1_w"], inputs["W1_b"]).astype(bf),
        "w2x": wx(inputs["W2_w"], inputs["W2_b"]).astype(bf),
        "w3x": wx(inputs["W3_w"], inputs["W3_b"]).astype(bf),
        "q1x": np.ascontiguousarray(
            np.vstack([np.asarray(inputs["Q_w1"], f),
                       np.asarray(inputs["Q_b1"], f)[None, :]])
        ),
        "q2x": np.ascontiguousarray(
            np.vstack([np.asarray(inputs["Q_w2"], f),
                       np.asarray(inputs["Q_b2"], f)[None, :]])
        ),
    }

    # A blocks: [p, h, k, m, r'] = A_m[c*R + h*HC + r', k*P + p] * SA, fp8
    Acl8 = (A_cl * SA).astype(f8)
    Aue8 = (A_ue * SA).astype(f8)

    in_maps = []
    for c in range(M):
        rs = slice(c * R, (c + 1) * R)
        # [R, N] -> [h, r', k, p] -> [p, h, k, r']
        acl = Acl8[rs, :].reshape(2, HC, KT, P).transpose(3, 0, 2, 1)
        aue = Aue8[rs, :].reshape(2, HC, KT, P).transpose(3, 0, 2, 1)
        m = dict(common)
        m["Aall"] = np.ascontiguousarray(
            np.stack([acl, aue], axis=3))  # [P, 2, KT, 2, HC]
        in_maps.append(m)
    return in_maps


def kernel(**inputs):
    global LAST_EXEC_NS, LAST_PROFILE
    nc = _get_module()
    from concourse.bass_utils import run_bass_kernel_spmd

    in_maps = prep_in_maps(inputs)
    res = run_bass_kernel_spmd(nc, in_maps, core_ids=list(range(M)), trace=False)
    LAST_EXEC_NS = res.exec_time_ns
    LAST_PROFILE = res.profile_json
    return np.asarray(res.results[0]["out"], np.float32)
